# revision 1
# baseline (speedup 1.0000x reference)
"""Multi-head attention (B=4, S=2048, E=1024, H=16, D=64) on 8 trn2 cores.

Sharding: core c handles batch b=c//2 and head-group hg=c%2 (8 heads, 512
embed cols). QKV projection weights are column-sharded by head group so
attention is fully local per device.

Per-core plan (bf16 matmul operands, fp32 PSUM accumulation):
  - X loaded via gpsimd cast-DMA (fp32->bf16), transposed to XT[e,q] by PE
    matmul against identity; W loaded via gpsimd cast-DMA (bf16).
  - QT[d,q], KT[d,q] = W.T @ XT (+bias via DVE); V[s,d] (+bias via K=1 ones
    matmul), stored augmented [V | 1] per head.
  - scores^T[k,q] = KT.T @ QT per head-pair: two K=64 matmuls row-tiled at
    partition bases 0/64 (run concurrently in the PE array).
  - e = exp(0.125 * s) single ACT pass PSUM->SBUF (bf16 out).
  - ctxT_aug[65,q] += [V|1].T @ e ; row 64 = Z (softmax denominator).
  - output: DVE 32x32 block-transpose + per-q 1/Z scale (Z columns obtained
    via a tiny DRAM bounce) + block-permuted DMA to DRAM.
  - Emission: DMA queue ordered X0-3 / Wq / Wk / Wv / X4-15; minimal
    prologue (4 transposes + first Q/K chunk) starts the exp stream ~7us
    in. The remaining transposes, all 16 V chains and Q/K chunks 1-3 are
    injected just-in-time inside head-pair 0's first q-chunk (after each
    k-tile's scores/exp so the score stream never blocks); later pairs'
    Q/K projections interleave between pairs. Transposes share the
    projection PSUM pool (pjp 2 + scores 4 + ctx 2 = 8 banks).
  - Output epilogue per unit: ctx drains + single Z bounce to DRAM first,
    both DVE block-transposes run during the Z round-trip, then
    reciprocal + scale + 4 HWDGE block-permuted stores.

Measured: ~390us/core (baseline 445us), rel err 3.9e-03 vs fp32 reference.
"""

import numpy as np
from contextlib import ExitStack

import concourse.bass as bass
import concourse.mybir as mybir
import concourse.tile as tile
from concourse.bass import ts, ds
from concourse.masks import make_identity

B, S, E = 4, 2048, 1024
H, DH = 16, 64
NCORES = 8
HG = 2                # head groups per batch (cores per batch element)
HPC = H // HG         # heads per core = 8
CE = HPC * DH         # embed cols per core = 512
P = 128
NQT = S // P          # 16 q-tiles of 128
QC = 4                # q chunks of 512
ET = E // P           # 8 e-tiles
MT = CE // P          # 4 output dim tiles (head pairs)

F32 = mybir.dt.float32
BF16 = mybir.dt.bfloat16
AF = mybir.ActivationFunctionType


def _build(tc, out, hs, wq, bq, wk, bk, wv, bv):
    nc = tc.nc
    with ExitStack() as ctx:
        persist = ctx.enter_context(tc.tile_pool(name="persist", bufs=1))
        xtp = ctx.enter_context(tc.tile_pool(name="xt_pool", bufs=1))
        xsp = ctx.enter_context(tc.tile_pool(name="xs_pool", bufs=16))
        ep = ctx.enter_context(tc.tile_pool(name="e_pool", bufs=12))
        cp = ctx.enter_context(tc.tile_pool(name="c_pool", bufs=4))
        otp = ctx.enter_context(tc.tile_pool(name="ot_pool", bufs=4))
        zp = ctx.enter_context(tc.tile_pool(name="z_pool", bufs=2))
        drp = ctx.enter_context(tc.tile_pool(name="dram_pool", bufs=2, space="DRAM"))
        pjp = ctx.enter_context(tc.tile_pool(name="proj_psum", bufs=2, space="PSUM"))

        # ---- persistent buffers ----
        qt = [persist.tile([P, S], BF16, tag=f"qt{m}", name=f"qt{m}")
              for m in range(MT)]
        kt = [persist.tile([P, S], BF16, tag=f"kt{m}", name=f"kt{m}")
              for m in range(MT)]
        v = [persist.tile([P, HPC, DH + 1], BF16, tag=f"v{st}", name=f"v{st}")
             for st in range(NQT)]
        bqs = persist.tile([P, MT], F32, tag="bqs")
        bks = persist.tile([P, MT], F32, tag="bks")
        nc.sync.dma_start(bqs, bq.rearrange("(o p) -> p o", p=P))
        nc.sync.dma_start(bks, bk.rearrange("(o p) -> p o", p=P))
        bvrow = persist.tile([1, CE], BF16, tag="bvrow")
        nc.gpsimd.dma_start(bvrow, bv[None, :])
        ones_row = persist.tile([1, P], BF16, tag="ones_row")
        nc.vector.memset(ones_row, 1.0)
        ones_col = persist.tile([P, HPC], BF16, tag="ones_col")
        nc.vector.memset(ones_col, 1.0)
        ident = persist.tile([P, P], BF16, tag="ident")
        make_identity(nc, ident)
        ws = {}
        for nm in ("wq", "wk", "wv"):
            ws[nm] = persist.tile([P, ET, CE], BF16, tag=nm, name=nm)

        def load_w(nm, wsrc):
            nc.gpsimd.dma_start(ws[nm], wsrc.rearrange("(o p) c -> p o c", p=P))

        xtall = xtp.tile([P, ET, S], BF16, tag="xtall", name="xtall")
        xt = [xtall[:, e] for e in range(ET)]

        hsr = hs.rearrange("(t p) e -> p t e", p=P)  # [128, 16, 1024]

        def load_x(qt_i):
            """Queue the cast-DMA for X tile qt_i."""
            xs_t = xsp.tile([P, E], BF16, tag="xs", name="xs")
            nc.gpsimd.dma_start(xs_t, hsr[:, qt_i, :])
            return xs_t

        xs_tiles = {}

        def transpose_x(qt_i, trp):
            """PE-transpose X tile qt_i into the xt tiles."""
            xs_t = xs_tiles.pop(qt_i)
            for eg in range(2):
                tp = trp.tile([P, 4, P], F32, tag="pps", name="pps")
                for j in range(4):
                    e = eg * 4 + j
                    nc.tensor.matmul(
                        tp[:, j, :], lhsT=xs_t[:, ts(e, P)], rhs=ident,
                        start=True, stop=True,
                    )
                nc.vector.tensor_copy(
                    out=xtall[:, ts(eg, 4), ts(qt_i, P)], in_=tp
                )

        def v_proj(st):
            """V projection for s-tile st (+bias via K=1 matmul), augmented."""
            ps = pjp.tile([P, 512], F32, tag="pps", name="pps")
            for e in range(ET):
                nc.tensor.matmul(
                    ps,
                    lhsT=xt[e][:, ts(st, P)],
                    rhs=ws["wv"][:, e, :],
                    start=(e == 0),
                    stop=False,
                )
            nc.tensor.matmul(ps, lhsT=ones_row, rhs=bvrow, start=False, stop=True)
            nc.vector.tensor_copy(
                out=v[st][:, :, 0:DH],
                in_=ps.rearrange("p (h d) -> p h d", h=HPC),
            )
            nc.vector.tensor_copy(out=v[st][:, :, DH], in_=ones_col)

        def qk_proj(m, qc):
            """Q and K projections for dim-tile m (head pair m), q-chunk qc."""
            for wname, dstt, bias in (("wq", qt, bqs), ("wk", kt, bks)):
                ps = pjp.tile([P, 512], F32, tag="pps", name="pps")
                for e in range(ET):
                    nc.tensor.matmul(
                        ps,
                        lhsT=ws[wname][:, e, ts(m, P)],
                        rhs=xt[e][:, ts(qc, 512)],
                        start=(e == 0),
                        stop=(e == ET - 1),
                    )
                nc.vector.tensor_scalar_add(
                    dstt[m][:, ts(qc, 512)], ps, bias[:, ts(m, 1)]
                )

        def attention_pair(pr, fill=None):
            """Full attention for head pair pr (heads 2pr, 2pr+1).

            fill: optional dict {(qc, kti): [thunk, ...]} of extra work
            emitted at the top of that k-tile iteration (pair-0 pipelining).
            """
            hA, hB = 2 * pr, 2 * pr + 1
            for qc in range(QC):
                st_lag = 1
                ctxA = cpp.tile([DH + 1, 512], F32, tag="ctx", name="ctx")
                ctxB = cpp.tile([DH + 1, 512], F32, tag="ctx", name="ctx")
                ets = {}
                for kti in range(NQT + st_lag):
                    if kti < NQT:
                        sps = spp.tile([P, 1024], F32, tag="sps", name="sps")
                        nc.tensor.matmul(
                            sps[:, 0:512],
                            lhsT=kt[pr][0:DH, ts(kti, P)],
                            rhs=qt[pr][0:DH, ts(qc, 512)],
                            start=True, stop=True,
                        )
                        nc.tensor.matmul(
                            sps[:, 512:1024],
                            lhsT=kt[pr][DH:P, ts(kti, P)],
                            rhs=qt[pr][DH:P, ts(qc, 512)],
                            start=True, stop=True,
                        )
                        et = ep.tile([P, 1024], BF16, tag="expT", name="expT")
                        nc.scalar.activation(et, sps, AF.Exp, scale=0.125)
                    # ctx staggered st_lag k-tiles behind so the PE never
                    # waits on the exp (deeper in pair-0 qc0 to absorb the
                    # just-in-time V-projection chains)
                    if fill is not None:
                        for thunk in fill.get((qc, kti), ()):
                            thunk()
                    pk = kti - st_lag
                    if pk >= 0:
                        pe = ets.pop(pk)
                        nc.tensor.matmul(
                            ctxA, lhsT=v[pk][:, hA, :], rhs=pe[:, 0:512],
                            start=(pk == 0), stop=(pk == NQT - 1),
                        )
                        nc.tensor.matmul(
                            ctxB, lhsT=v[pk][:, hB, :], rhs=pe[:, 512:1024],
                            start=(pk == 0), stop=(pk == NQT - 1),
                        )
                    if kti < NQT:
                        ets[kti] = et

                # normalize + transpose + store via DVE/DMA (no PE).
                # Order: cs copies + zd writes first, then both block
                # transposes (DVE busy during the zd->c2 DRAM round-trip),
                # then reciprocal + scales + stores.
                zd = drp.tile([2, 2, 512], F32, tag="zd", name="zd")
                css = []
                for idx, ctx_ps in enumerate((ctxA, ctxB)):
                    cs = cp.tile([DH + 1, 512], F32, tag="cs", name="cs")
                    nc.vector.tensor_copy(out=cs, in_=ctx_ps)
                    nc.sync.dma_start(zd[0, idx][None, :], cs[DH : DH + 1, :])
                    css.append(cs)
                c2 = zp.tile([DH, 2, NQT], F32, tag="c2", name="c2")
                for i in range(2):
                    nc.sync.dma_start(
                        c2[ts(i, 32)],
                        zd[0].rearrange("h (j a) -> a h j", a=32),
                    )
                bts = []
                for idx in range(2):
                    bt = otp.tile([DH, 512], F32, tag="bt", name="bt")
                    nc.vector.transpose(bt, css[idx][0:DH, :])
                    bts.append(bt)
                nc.vector.reciprocal(c2, c2)
                for idx, hl in ((0, hA), (1, hB)):
                    ot = otp.tile([DH, NQT, 32], F32, tag="ot", name="ot")
                    nc.vector.tensor_tensor(
                        ot,
                        bts[idx].rearrange("p (j b) -> p j b", b=32),
                        c2[:, idx, :, None].to_broadcast([DH, NQT, 32]),
                        mybir.AluOpType.mult,
                    )
                    # block-permuted store: ot[32i+a, j, b] -> row qc*512+32j+a,
                    # col hl*64+32i+b
                    for i in range(2):
                        nc.sync.dma_start(
                            out.rearrange(
                                "(qq j a) (h i b) -> qq h i a j b",
                                j=NQT, a=32, i=2, b=32,
                            )[qc, hl, i],
                            ot[ts(i, 32)],
                        )

        # ---- emission: DMA order X0-3, weights, X4-15; minimal prologue
        # (4 transposes + first Q/K chunk) so the exp stream starts ~7us in;
        # remaining transposes, V chains and Q/K chunks are injected
        # just-in-time inside pair 0's first q-chunk (after each k-tile's
        # scores/exp so the score stream never blocks) ----
        for qt_i in range(4):
            xs_tiles[qt_i] = load_x(qt_i)
        load_w("wq", wq)
        load_w("wk", wk)
        load_w("wv", wv)
        for qt_i in range(4, NQT):
            xs_tiles[qt_i] = load_x(qt_i)

        spp = ctx.enter_context(tc.tile_pool(name="s_psum", bufs=2, space="PSUM"))
        cpp = ctx.enter_context(tc.tile_pool(name="ctx_psum", bufs=2, space="PSUM"))

        for qt_i in range(4):
            transpose_x(qt_i, pjp)
        qk_proj(0, 0)

        fill0 = {}

        def add_fill(qc, kti, thunk):
            fill0.setdefault((qc, kti), []).append(thunk)

        sched = {
            1: [lambda: transpose_x(4, pjp), lambda: transpose_x(5, pjp)],
            2: [lambda: transpose_x(6, pjp), lambda: transpose_x(7, pjp)],
            3: [lambda: qk_proj(0, 1)],
            5: [lambda: transpose_x(8, pjp), lambda: transpose_x(9, pjp)],
            6: [lambda: transpose_x(10, pjp), lambda: transpose_x(11, pjp)],
            7: [lambda: qk_proj(0, 2)],
            9: [lambda: transpose_x(12, pjp), lambda: transpose_x(13, pjp)],
            10: [lambda: transpose_x(14, pjp), lambda: transpose_x(15, pjp)],
            11: [lambda: qk_proj(0, 3)],
        }
        for kti, thunks in sched.items():
            for t in thunks:
                add_fill(0, kti, t)
        # V chains just-in-time: ctx for k-tile pk runs at iteration pk+1 of
        # qc=0, so emit v_proj(pk) at iteration pk (before its first use).
        for st in range(NQT):
            add_fill(0, st, (lambda z: lambda: v_proj(z))(st))

        attention_pair(0, fill=fill0)
        for pr in range(1, MT):
            for qc in range(QC):
                qk_proj(pr, qc)
            attention_pair(pr)


def build_program():
    from concourse import bacc

    nc = bacc.Bacc("TRN2", target_bir_lowering=False, debug=False)
    hs = nc.dram_tensor("hs", [S, E], F32, kind="ExternalInput").ap()
    wq = nc.dram_tensor("wq", [E, CE], F32, kind="ExternalInput").ap()
    bq = nc.dram_tensor("bq", [CE], F32, kind="ExternalInput").ap()
    wk = nc.dram_tensor("wk", [E, CE], F32, kind="ExternalInput").ap()
    bk = nc.dram_tensor("bk", [CE], F32, kind="ExternalInput").ap()
    wv = nc.dram_tensor("wv", [E, CE], F32, kind="ExternalInput").ap()
    bv = nc.dram_tensor("bv", [CE], F32, kind="ExternalInput").ap()
    out = nc.dram_tensor("out", [S, CE], F32, kind="ExternalOutput").ap()
    with tile.TileContext(nc) as tc:
        _build(tc, out, hs, wq, bq, wk, bk, wv, bv)
    nc.compile()
    return nc


def make_in_maps(inputs):
    """Slice full inputs into 8 per-core input maps."""
    hsf = np.ascontiguousarray(np.asarray(inputs["hidden_states"], dtype=np.float32))
    w = {k: np.asarray(inputs[k], dtype=np.float32) for k in
         ("Wq", "bq", "Wk", "bk", "Wv", "bv")}
    in_maps = []
    for core in range(NCORES):
        b, hg = core // HG, core % HG
        cols = slice(hg * CE, (hg + 1) * CE)
        in_maps.append({
            "hs": hsf[b],
            "wq": np.ascontiguousarray(w["Wq"][:, cols]),
            "bq": np.ascontiguousarray(w["bq"][cols]),
            "wk": np.ascontiguousarray(w["Wk"][:, cols]),
            "bk": np.ascontiguousarray(w["bk"][cols]),
            "wv": np.ascontiguousarray(w["Wv"][:, cols]),
            "bv": np.ascontiguousarray(w["bv"][cols]),
        })
    return in_maps


def assemble(results):
    """Gather 8 per-core [S, CE] outputs into the full [B, S, E] output."""
    full = np.empty((B, S, E), dtype=np.float32)
    for core in range(NCORES):
        b, hg = core // HG, core % HG
        full[b, :, hg * CE : (hg + 1) * CE] = results[core]["out"]
    return full


_NC_CACHE = None


def kernel(**inputs):
    global _NC_CACHE
    from concourse.bass_utils import run_bass_kernel_spmd

    if _NC_CACHE is None:
        _NC_CACHE = build_program()
    res = run_bass_kernel_spmd(_NC_CACHE, make_in_maps(inputs),
                               core_ids=list(range(NCORES)))
    return assemble(res.results)

